# revision 1
# baseline (speedup 1.0000x reference)
"""Ball-query kernel for Trainium2 (Bass/Tile), 8 NeuronCores.

Problem: for each batch b (8 total) and each query point m (4096), return the
first 32 source indices n (in increasing n) with ||q_m - p_n||^2 < 0.2^2,
padding unused slots with the first valid index. Queries == sources (xyz).

Sharding: data-parallel over batch, one batch per core (8 cores).

Per-core algorithm (N=4096 queries x 4096 sources):
  - PE computes dot[m, n] = q_m . p_n per 128-query block (K=3 matmul).
  - DVE scalar_tensor_tensor: s = 2*dot - plus, where plus[m,n] = sq[m]+sq[n]
    (s == -d2 with bit-exact match to the reference's rounding order).
  - DVE STT: keys = (s > -r^2) * (4096 - n)  -> valid keys descending encode
    ascending indices; invalid -> 0.
  - 4 rounds of vector.max (top-8, descending) + match_replace to extract the
    32 largest keys = first 32 valid indices, in order.
  - Pad empty slots (key 0) with the first valid key; idx = 4096 - key.
"""

import numpy as np

N = 4096
NS = 32
R2 = 0.04
NCORES = 8
BLK = 128
NBLK = N // BLK   # 32
CH = 2048         # psum chunk (4 banks)
NCH = N // CH     # 2
MM = 512          # matmul free-dim per instruction (1 bank)


def _build_bass():
    import concourse.bass as bass
    import concourse.mybir as mybir
    from concourse import bacc, tile

    Alu = mybir.AluOpType
    f32 = mybir.dt.float32

    nc = bacc.Bacc(
        "TRN2", target_bir_lowering=False, debug=False, num_devices=NCORES
    )

    xyzT_d = nc.dram_tensor("xyzT", [3, N], f32, kind="ExternalInput")
    # sqA = [sqrep | sqq | inegrep]: per-row [sq(n) x N, sq_q blocks x 32, 4096-n x N]
    sqA_d = nc.dram_tensor("sqA", [128, 2 * N + NBLK], f32, kind="ExternalInput")
    out_d = nc.dram_tensor("out", [N, NS], mybir.dt.int32, kind="ExternalOutput")

    with tile.TileContext(nc) as tc:
        with (
            tc.tile_pool(name="const", bufs=1) as cpool,
            tc.tile_pool(name="psum", bufs=8, space="PSUM") as ppool,
            tc.tile_pool(name="work", bufs=2) as wpool,
            tc.tile_pool(name="small", bufs=3) as spool,
        ):
            xyzT_sb = cpool.tile([3, N], f32, tag="xyzT", name="xyzT_sb")
            nc.gpsimd.dma_start(xyzT_sb[:], xyzT_d.ap())
            sqA_sb = cpool.tile([128, 2 * N + NBLK], f32, tag="sqA", name="sqA_sb")
            nc.gpsimd.dma_start(sqA_sb[:], sqA_d.ap())
            def sqrep_sl(lo, hi):
                return sqA_sb[:, lo:hi]

            def sqq_sl(b):
                return sqA_sb[:, N + b : N + b + 1]

            def ineg_sl(lo, hi):
                return sqA_sb[:, N + NBLK + lo : N + NBLK + hi]

            for b in range(NBLK):
                # plus[m, n] = sq_q[m] + sq_src[n]
                plus = wpool.tile([128, N], f32, tag="plus", name="plus")
                for c in range(NCH):
                    nc.vector.tensor_scalar(
                        plus[:, c * CH : (c + 1) * CH],
                        sqrep_sl(c * CH, (c + 1) * CH),
                        sqq_sl(b),
                        None,
                        Alu.add,
                    )

                keys = wpool.tile([128, N], f32, tag="keys", name="keys")
                keys2 = wpool.tile([128, N], f32, tag="keys2", name="keys2")

                for j in range(N // MM):
                    ps = ppool.tile([128, MM], f32, tag="ps", name="ps")
                    nc.tensor.matmul(
                        ps[:],
                        xyzT_sb[:, b * BLK : (b + 1) * BLK],
                        xyzT_sb[:, j * MM : (j + 1) * MM],
                        start=True,
                        stop=True,
                    )
                    # s = 2*dot - plus  (== -d2, exact)
                    nc.vector.scalar_tensor_tensor(
                        keys2[:, j * MM : (j + 1) * MM],
                        ps[:],
                        2.0,
                        plus[:, j * MM : (j + 1) * MM],
                        Alu.mult,
                        Alu.subtract,
                    )
                for c in range(NCH):
                    # keys = (s > -r2) * (4096 - n)
                    nc.vector.scalar_tensor_tensor(
                        keys[:, c * CH : (c + 1) * CH],
                        keys2[:, c * CH : (c + 1) * CH],
                        -R2,
                        ineg_sl(c * CH, (c + 1) * CH),
                        Alu.is_gt,
                        Alu.mult,
                    )

                v8 = spool.tile([128, NS], f32, tag="v8", name="v8")
                nc.vector.max(v8[:, 0:8], keys[:])
                nc.vector.match_replace(keys2[:], v8[:, 0:8], keys[:], 0.0)
                nc.vector.max(v8[:, 8:16], keys2[:])
                nc.vector.match_replace(keys[:], v8[:, 8:16], keys2[:], 0.0)
                nc.vector.max(v8[:, 16:24], keys[:])
                nc.vector.match_replace(keys2[:], v8[:, 16:24], keys[:], 0.0)
                nc.vector.max(v8[:, 24:32], keys2[:])

                # pad empty slots (0) with first valid key, then idx = 4096 - key
                f8 = spool.tile([128, NS], f32, tag="f8", name="f8")
                nc.vector.tensor_scalar(f8[:], v8[:], 0.0, None, Alu.is_equal)
                t2 = spool.tile([128, NS], f32, tag="t2", name="t2")
                nc.vector.scalar_tensor_tensor(
                    t2[:], f8[:], v8[:, 0:1], v8[:], Alu.mult, Alu.add
                )
                idx = spool.tile([128, NS], mybir.dt.int32, tag="idx", name="idx")
                nc.vector.tensor_scalar(
                    idx[:], t2[:], -1.0, float(N), Alu.mult, Alu.add
                )
                nc.sync.dma_start(
                    out_d.ap()[b * BLK : (b + 1) * BLK, :], idx[:]
                )

    nc.compile()
    return nc


def kernel(xyz, xyz_new=None):
    from concourse.bass_utils import run_bass_kernel_spmd

    xyz = np.asarray(xyz, dtype=np.float32)
    nc = _build_bass()

    iota_neg = (np.float32(N) - np.arange(N, dtype=np.float32)).astype(np.float32)
    in_maps = []
    for b in range(NCORES):
        P = xyz[b]  # [4096, 3]
        x, y, z = P[:, 0], P[:, 1], P[:, 2]
        sq = (x * x + y * y) + z * z  # fp32, reference order
        row = np.concatenate([sq, np.zeros(NBLK, np.float32), iota_neg])
        sqA = np.broadcast_to(row, (128, 2 * N + NBLK)).copy()
        sqA[:, N : N + NBLK] = sq.reshape(NBLK, 128).T
        in_maps.append(
            {
                "xyzT": np.ascontiguousarray(P.T),
                "sqA": sqA,
            }
        )

    import os

    trace = bool(int(os.environ.get("BQ_TRACE", "0")))
    try:
        res = run_bass_kernel_spmd(
            nc, in_maps, core_ids=list(range(NCORES)), trace=trace
        )
    except ModuleNotFoundError:
        res = run_bass_kernel_spmd(nc, in_maps, core_ids=list(range(NCORES)))
    if trace and res.exec_time_ns is not None:
        print(f"HW exec time: {res.exec_time_ns} ns")
    return np.stack([res.results[b]["out"] for b in range(NCORES)]).astype(np.int32)


if __name__ == "__main__":
    rng = np.random.default_rng(0)
    xyz = rng.random((8, N, 3), dtype=np.float32)
    out = kernel(xyz)
    print(out.shape, out.dtype)



# revision 2
# speedup vs baseline: 10.3896x; 10.3896x over previous
"""Ball-query kernel for Trainium2 (Bass/Tile), 8 NeuronCores.

Problem: for each batch b (8 total) and each query point m (4096), return the
first 32 source indices n (in increasing n) with ||q_m - p_n||^2 < 0.2^2,
padding unused slots with the first valid index. Queries == sources (xyz).

Sharding: data-parallel over batch, one batch per core (8 cores).

Per-core algorithm (N=4096 queries x 4096 sources):
  - PE computes dot[m, n] = q_m . p_n per 128-query block (K=3 matmul).
  - DVE scalar_tensor_tensor: s = 2*dot - plus, where plus[m,n] = sq[m]+sq[n]
    (s == -d2 with bit-exact match to the reference's rounding order).
  - DVE STT: keys = (s > -r^2) * (4096 - n)  -> valid keys descending encode
    ascending indices; invalid -> 0.
  - 4 rounds of vector.max (top-8, descending) + match_replace to extract the
    32 largest keys = first 32 valid indices, in order.
  - Pad empty slots (key 0) with the first valid key; idx = 4096 - key.

Host I/O is the bottleneck (axon-tunneled PJRT), so the host->device payload
is a single [5, 4096] f32 tensor per core (xyzT rows, sq row, sqq row); the
[128, N] broadcasts are materialized on-device (stride-0 DMA + iota), the
output travels as int16, and the jitted executable + donated output buffer
are cached across kernel() calls.
"""

import numpy as np

N = 4096
NS = 32
R2 = 0.04
NCORES = 8
BLK = 128
NBLK = N // BLK   # 32
CH = 2048         # dve chunk
NCH = N // CH     # 2
MM = 512          # matmul free-dim per instruction (1 bank)

_CACHE = {}


def _build_bass():
    import concourse.bass as bass
    import concourse.mybir as mybir
    from concourse import bacc, tile

    Alu = mybir.AluOpType
    f32 = mybir.dt.float32
    i16 = mybir.dt.int16

    nc = bacc.Bacc(
        "TRN2", target_bir_lowering=False, debug=False, num_devices=NCORES
    )

    # rows 0:3 = xyzT; row 3 = sq (source order); row 4 = sqq flat (p*32+b)
    inp_d = nc.dram_tensor("inp", [5, N], f32, kind="ExternalInput")
    out_d = nc.dram_tensor("out", [N, NS], i16, kind="ExternalOutput")

    with tile.TileContext(nc) as tc:
        with (
            tc.tile_pool(name="const", bufs=1) as cpool,
            tc.tile_pool(name="psum", bufs=8, space="PSUM") as ppool,
            tc.tile_pool(name="work", bufs=2) as wpool,
            tc.tile_pool(name="small", bufs=3) as spool,
        ):
            xyzT_sb = cpool.tile([3, N], f32, tag="xyzT", name="xyzT_sb")
            nc.gpsimd.dma_start(xyzT_sb[:], inp_d.ap()[0:3, :])
            # sq broadcast along partitions via stride-0 DMA read
            sqrep = cpool.tile([128, N], f32, tag="sqrep", name="sqrep")
            nc.gpsimd.dma_start(
                sqrep[:], inp_d.ap()[3:4, :].partition_broadcast(128).squeeze(1)
            )
            # sqq[p, b] = sq[b*128 + p], packed row-major as row4[p*32 + b]
            sqq_sb = cpool.tile([128, NBLK], f32, tag="sqq", name="sqq_sb")
            nc.gpsimd.dma_start(
                sqq_sb[:], inp_d.ap()[4:5, :].rearrange("a (p c) -> (a p) c", p=128)
            )
            # ineg[p, n] = 4096 - n (exact in f32)
            ineg = cpool.tile([128, N], f32, tag="ineg", name="ineg")
            nc.gpsimd.iota(
                ineg[:],
                pattern=[[-1, N]],
                base=N,
                channel_multiplier=0,
                allow_small_or_imprecise_dtypes=True,
            )

            for b in range(NBLK):
                # plus[m, n] = sq_q[m] + sq_src[n]
                plus = wpool.tile([128, N], f32, tag="plus", name="plus")
                for c in range(NCH):
                    nc.vector.tensor_scalar(
                        plus[:, c * CH : (c + 1) * CH],
                        sqrep[:, c * CH : (c + 1) * CH],
                        sqq_sb[:, b : b + 1],
                        None,
                        Alu.add,
                    )

                keys = wpool.tile([128, N], f32, tag="keys", name="keys")
                keys2 = wpool.tile([128, N], f32, tag="keys2", name="keys2")

                for j in range(N // MM):
                    ps = ppool.tile([128, MM], f32, tag="ps", name="ps")
                    nc.tensor.matmul(
                        ps[:],
                        xyzT_sb[:, b * BLK : (b + 1) * BLK],
                        xyzT_sb[:, j * MM : (j + 1) * MM],
                        start=True,
                        stop=True,
                    )
                    # s = 2*dot - plus  (== -d2, exact)
                    nc.vector.scalar_tensor_tensor(
                        keys2[:, j * MM : (j + 1) * MM],
                        ps[:],
                        2.0,
                        plus[:, j * MM : (j + 1) * MM],
                        Alu.mult,
                        Alu.subtract,
                    )
                for c in range(NCH):
                    # keys = (s > -r2) * (4096 - n)
                    nc.vector.scalar_tensor_tensor(
                        keys[:, c * CH : (c + 1) * CH],
                        keys2[:, c * CH : (c + 1) * CH],
                        -R2,
                        ineg[:, c * CH : (c + 1) * CH],
                        Alu.is_gt,
                        Alu.mult,
                    )

                v8 = spool.tile([128, NS], f32, tag="v8", name="v8")
                nc.vector.max(v8[:, 0:8], keys[:])
                nc.vector.match_replace(keys2[:], v8[:, 0:8], keys[:], 0.0)
                nc.vector.max(v8[:, 8:16], keys2[:])
                nc.vector.match_replace(keys[:], v8[:, 8:16], keys2[:], 0.0)
                nc.vector.max(v8[:, 16:24], keys[:])
                nc.vector.match_replace(keys2[:], v8[:, 16:24], keys[:], 0.0)
                nc.vector.max(v8[:, 24:32], keys2[:])

                # pad empty slots (0) with first valid key, then idx = 4096 - key
                f8 = spool.tile([128, NS], f32, tag="f8", name="f8")
                nc.vector.tensor_scalar(f8[:], v8[:], 0.0, None, Alu.is_equal)
                t2 = spool.tile([128, NS], f32, tag="t2", name="t2")
                nc.vector.scalar_tensor_tensor(
                    t2[:], f8[:], v8[:, 0:1], v8[:], Alu.mult, Alu.add
                )
                idx = spool.tile([128, NS], i16, tag="idx", name="idx")
                nc.vector.tensor_scalar(
                    idx[:], t2[:], -1.0, float(N), Alu.mult, Alu.add
                )
                nc.sync.dma_start(
                    out_d.ap()[b * BLK : (b + 1) * BLK, :], idx[:]
                )

    nc.compile()
    return nc


def _init():
    import jax
    from jax.sharding import Mesh, PartitionSpec, NamedSharding

    try:
        from jax.experimental.shard_map import shard_map
    except ImportError:
        from jax import shard_map
    import concourse.mybir as mybir
    from concourse.bass2jax import (
        _bass_exec_p,
        install_neuronx_cc_hook,
        partition_id_tensor,
    )

    install_neuronx_cc_hook()
    nc = _build_bass()

    partition_name = (
        nc.partition_id_tensor.name if nc.partition_id_tensor else None
    )
    in_names, out_names, out_avals = [], [], []
    for alloc in nc.m.functions[0].allocations:
        if not isinstance(alloc, mybir.MemoryLocationSet):
            continue
        name = alloc.memorylocations[0].name
        if alloc.kind == "ExternalInput":
            if name != partition_name:
                in_names.append(name)
        elif alloc.kind == "ExternalOutput":
            shape = tuple(alloc.tensor_shape)
            dtype = mybir.dt.np(alloc.dtype)
            out_names.append(name)
            out_avals.append(jax.core.ShapedArray(shape, dtype))
    n_params = len(in_names)
    n_outs = len(out_avals)
    in_names_full = in_names + out_names + (
        [partition_name] if partition_name else []
    )
    donate = tuple(range(n_params, n_params + n_outs))

    def _body(*args):
        operands = list(args)
        if partition_name is not None:
            operands.append(partition_id_tensor())
        outs = _bass_exec_p.bind(
            *operands,
            out_avals=tuple(out_avals),
            in_names=tuple(in_names_full),
            out_names=tuple(out_names),
            lowering_input_output_aliases=(),
            sim_require_finite=True,
            sim_require_nnan=True,
            nc=nc,
        )
        return tuple(outs)

    devices = jax.devices()[:NCORES]
    mesh = Mesh(np.asarray(devices), ("core",))
    sh = NamedSharding(mesh, PartitionSpec("core"))
    in_specs = (PartitionSpec("core"),) * (n_params + n_outs)
    out_specs = (PartitionSpec("core"),) * n_outs
    fn = jax.jit(
        shard_map(
            _body, mesh=mesh, in_specs=in_specs, out_specs=out_specs,
            check_rep=False,
        ),
        donate_argnums=donate,
        keep_unused=True,
    )
    _CACHE.update(
        jax=jax, fn=fn, sh=sh, prev=None,
        out_shape=[(NCORES * N, NS)], out_dtype=[np.int16],
    )
    return _CACHE


def _prep(xyz):
    # [8, 4096, 3] -> concat of per-core [5, 4096]: xyzT rows, sq row, sqq row
    xyz = np.ascontiguousarray(np.asarray(xyz, dtype=np.float32))
    x, y, z = xyz[..., 0], xyz[..., 1], xyz[..., 2]
    sq = (x * x + y * y) + z * z                      # [8, 4096] reference order
    inp = np.empty((NCORES, 5, N), np.float32)
    inp[:, 0:3, :] = xyz.transpose(0, 2, 1)
    inp[:, 3, :] = sq
    inp[:, 4, :] = sq.reshape(NCORES, NBLK, BLK).transpose(0, 2, 1).reshape(
        NCORES, N
    )
    return inp.reshape(NCORES * 5, N)


def kernel(xyz, xyz_new=None):
    st = _CACHE if _CACHE else _init()
    jax, fn, sh = st["jax"], st["fn"], st["sh"]

    inp_dev = jax.device_put(_prep(xyz), sh)
    prev = st["prev"]
    if prev is None:
        prev = [
            jax.device_put(np.zeros(s, d), sh)
            for s, d in zip(st["out_shape"], st["out_dtype"])
        ]
    outs = fn(inp_dev, *prev)
    res = np.asarray(outs[0])
    st["prev"] = list(outs)
    return res.reshape(NCORES, N, NS).astype(np.int32)


if __name__ == "__main__":
    rng = np.random.default_rng(0)
    xyz = rng.random((8, N, 3), dtype=np.float32)
    out = kernel(xyz)
    print(out.shape, out.dtype)


# revision 8
# speedup vs baseline: 11.6020x; 1.1167x over previous
"""Ball-query kernel for Trainium2 (Bass/Tile), 8 NeuronCores.

Problem: for each batch b (8 total) and each query point m (4096), return the
first 32 source indices n (in increasing n) with ||q_m - p_n||^2 < 0.2^2,
padding unused slots with the first valid index. Queries == sources (xyz).

Sharding: data-parallel over batch, one batch per core (8 cores).

Per-core algorithm (N=4096 queries x 4096 sources):
  - PE computes dot[m, n] = q_m . p_n per 128-query block (K=3 matmul).
  - DVE scalar_tensor_tensor: s = 2*dot - plus, where plus[m,n] = sq[m]+sq[n]
    (s == -d2 with bit-exact match to the reference's rounding order).
  - DVE STT: keys = (s > -r^2) * (4096 - n)  -> valid keys descending encode
    ascending indices; invalid -> 0.
  - 4 rounds of vector.max (top-8, descending) + match_replace to extract the
    32 largest keys = first 32 valid indices, in order.
  - Pad empty slots (key 0) with the first valid key; idx = 4096 - key.

Host I/O is the bottleneck (axon-tunneled PJRT), so the host->device payload
is a single [4, 4096] f32 tensor per core (xyzT rows + sq row; sqq and the
[128, N] broadcasts are materialized on-device via rearranged/stride-0 DMA +
iota), index pairs travel back packed as 24-bit triples in a uint8 tensor
(idx_even + 4096*idx_odd, exact in f32 since it is < 2^24), and the jitted
executable + donated output buffer are cached across kernel() calls.
"""

import numpy as np

N = 4096
NS = 32
R2 = 0.04
NCORES = 8
BLK = 128
NBLK = N // BLK   # 32
CH = 2048         # dve chunk
NCH = N // CH     # 2
MM = 512          # matmul free-dim per instruction (1 bank)
NP2 = NS // 2     # 16 packed index pairs per query
NB3 = 3 * NP2     # 48 output bytes per query

_CACHE = {}


def _build_bass():
    import concourse.bass as bass
    import concourse.mybir as mybir
    from concourse import bacc, tile

    Alu = mybir.AluOpType
    f32 = mybir.dt.float32
    i32 = mybir.dt.int32
    u8 = mybir.dt.uint8

    nc = bacc.Bacc(
        "TRN2", target_bir_lowering=False, debug=False, num_devices=NCORES
    )

    # rows 0:3 = xyzT; row 3 = sq (source order)
    inp_d = nc.dram_tensor("inp", [4, N], f32, kind="ExternalInput")
    out_d = nc.dram_tensor("out", [N, NB3], u8, kind="ExternalOutput")

    with tile.TileContext(nc) as tc:
        with (
            tc.tile_pool(name="const", bufs=1) as cpool,
            tc.tile_pool(name="psum", bufs=8, space="PSUM") as ppool,
            tc.tile_pool(name="work", bufs=2) as wpool,
            tc.tile_pool(name="small", bufs=3) as spool,
        ):
            xyzT_sb = cpool.tile([3, N], f32, tag="xyzT", name="xyzT_sb")
            nc.gpsimd.dma_start(xyzT_sb[:], inp_d.ap()[0:3, :])
            # sq broadcast along partitions via stride-0 DMA read
            sqrep = cpool.tile([128, N], f32, tag="sqrep", name="sqrep")
            nc.gpsimd.dma_start(
                sqrep[:], inp_d.ap()[3:4, :].partition_broadcast(128).squeeze(1)
            )
            # sqq[p, b] = sq[b*128 + p]: partition-major reread of the sq row
            sqq_sb = cpool.tile([128, NBLK], f32, tag="sqq", name="sqq_sb")
            nc.gpsimd.dma_start(
                sqq_sb[:], inp_d.ap()[3:4, :].rearrange("a (c p) -> p (a c)", p=128)
            )
            # ineg[p, n] = 4096 - n (exact in f32)
            ineg = cpool.tile([128, N], f32, tag="ineg", name="ineg")
            nc.gpsimd.iota(
                ineg[:],
                pattern=[[-1, N]],
                base=N,
                channel_multiplier=0,
                allow_small_or_imprecise_dtypes=True,
            )

            for b in range(NBLK):
                # plus[m, n] = sq_q[m] + sq_src[n]
                plus = wpool.tile([128, N], f32, tag="plus", name="plus")
                for c in range(NCH):
                    nc.vector.tensor_scalar(
                        plus[:, c * CH : (c + 1) * CH],
                        sqrep[:, c * CH : (c + 1) * CH],
                        sqq_sb[:, b : b + 1],
                        None,
                        Alu.add,
                    )

                keys = wpool.tile([128, N], f32, tag="keys", name="keys")
                keys2 = wpool.tile([128, N], f32, tag="keys2", name="keys2")

                for j in range(N // MM):
                    ps = ppool.tile([128, MM], f32, tag="ps", name="ps")
                    nc.tensor.matmul(
                        ps[:],
                        xyzT_sb[:, b * BLK : (b + 1) * BLK],
                        xyzT_sb[:, j * MM : (j + 1) * MM],
                        start=True,
                        stop=True,
                    )
                    # s = 2*dot - plus  (== -d2, exact)
                    nc.vector.scalar_tensor_tensor(
                        keys2[:, j * MM : (j + 1) * MM],
                        ps[:],
                        2.0,
                        plus[:, j * MM : (j + 1) * MM],
                        Alu.mult,
                        Alu.subtract,
                    )
                for c in range(NCH):
                    # keys = (s > -r2) * (4096 - n)
                    nc.vector.scalar_tensor_tensor(
                        keys[:, c * CH : (c + 1) * CH],
                        keys2[:, c * CH : (c + 1) * CH],
                        -R2,
                        ineg[:, c * CH : (c + 1) * CH],
                        Alu.is_gt,
                        Alu.mult,
                    )

                v8 = spool.tile([128, NS], f32, tag="v8", name="v8")
                nc.vector.max(v8[:, 0:8], keys[:])
                nc.vector.match_replace(keys2[:], v8[:, 0:8], keys[:], 0.0)
                nc.vector.max(v8[:, 8:16], keys2[:])
                nc.vector.match_replace(keys[:], v8[:, 8:16], keys2[:], 0.0)
                nc.vector.max(v8[:, 16:24], keys[:])
                nc.vector.match_replace(keys2[:], v8[:, 16:24], keys[:], 0.0)
                nc.vector.max(v8[:, 24:32], keys2[:])

                # pad empty slots (0) with first valid key, then pack index
                # pairs: idx_even + 4096*idx_odd = 2^24 + 4096 - k_e - 4096*k_o
                # (exact in f32: result <= 2^24 - 1)
                f8 = spool.tile([128, NS], f32, tag="f8", name="f8")
                nc.vector.tensor_scalar(f8[:], v8[:], 0.0, None, Alu.is_equal)
                t2 = spool.tile([128, NS], f32, tag="t2", name="t2")
                nc.vector.scalar_tensor_tensor(
                    t2[:], f8[:], v8[:, 0:1], v8[:], Alu.mult, Alu.add
                )
                idxf = spool.tile([128, NS], f32, tag="idxf", name="idxf")
                nc.vector.tensor_scalar(
                    idxf[:], t2[:], -1.0, float(N), Alu.mult, Alu.add
                )
                packed = spool.tile([128, NP2], i32, tag="packed", name="packed")
                nc.vector.scalar_tensor_tensor(
                    packed[:], idxf[:, 1::2], 4096.0, idxf[:, 0::2],
                    Alu.mult, Alu.add,
                )
                # low 3 bytes of each int32 lane -> 3 output bytes (LE)
                src_b = packed[:].bitcast(u8).rearrange(
                    "p (j k) -> p j k", k=4
                )[:, :, 0:3]
                dst_b = out_d.ap()[b * BLK : (b + 1) * BLK, :].rearrange(
                    "p (j k) -> p j k", k=3
                )
                nc.sync.dma_start(dst_b, src_b)

    nc.compile()
    return nc


def _init():
    import jax
    from jax.sharding import Mesh, PartitionSpec, NamedSharding

    try:
        from jax.experimental.shard_map import shard_map
    except ImportError:
        from jax import shard_map
    import concourse.mybir as mybir
    from concourse.bass2jax import (
        _bass_exec_p,
        install_neuronx_cc_hook,
        partition_id_tensor,
    )

    install_neuronx_cc_hook()
    nc = _build_bass()

    partition_name = (
        nc.partition_id_tensor.name if nc.partition_id_tensor else None
    )
    in_names, out_names, out_avals = [], [], []
    for alloc in nc.m.functions[0].allocations:
        if not isinstance(alloc, mybir.MemoryLocationSet):
            continue
        name = alloc.memorylocations[0].name
        if alloc.kind == "ExternalInput":
            if name != partition_name:
                in_names.append(name)
        elif alloc.kind == "ExternalOutput":
            shape = tuple(alloc.tensor_shape)
            dtype = mybir.dt.np(alloc.dtype)
            out_names.append(name)
            out_avals.append(jax.core.ShapedArray(shape, dtype))
    n_params = len(in_names)
    n_outs = len(out_avals)
    in_names_full = in_names + out_names + (
        [partition_name] if partition_name else []
    )
    donate = tuple(range(n_params, n_params + n_outs))

    def _body(*args):
        operands = list(args)
        if partition_name is not None:
            operands.append(partition_id_tensor())
        outs = _bass_exec_p.bind(
            *operands,
            out_avals=tuple(out_avals),
            in_names=tuple(in_names_full),
            out_names=tuple(out_names),
            lowering_input_output_aliases=(),
            sim_require_finite=True,
            sim_require_nnan=True,
            nc=nc,
        )
        return tuple(outs)

    devices = jax.devices()[:NCORES]
    mesh = Mesh(np.asarray(devices), ("core",))
    sh = NamedSharding(mesh, PartitionSpec("core"))
    in_specs = (PartitionSpec("core"),) * (n_params + n_outs)
    out_specs = (PartitionSpec("core"),) * n_outs
    fn = jax.jit(
        shard_map(
            _body, mesh=mesh, in_specs=in_specs, out_specs=out_specs,
            check_rep=False,
        ),
        donate_argnums=donate,
        keep_unused=True,
    )
    _CACHE.update(
        jax=jax, fn=fn, sh=sh, prev=None,
        out_shape=[(NCORES * N, NB3)], out_dtype=[np.uint8],
    )
    return _CACHE


def _prep(xyz):
    # [8, 4096, 3] -> concat of per-core [4, 4096]: xyzT rows + sq row
    xyz = np.ascontiguousarray(np.asarray(xyz, dtype=np.float32))
    x, y, z = xyz[..., 0], xyz[..., 1], xyz[..., 2]
    sq = (x * x + y * y) + z * z                      # [8, 4096] reference order
    inp = np.empty((NCORES, 4, N), np.float32)
    inp[:, 0:3, :] = xyz.transpose(0, 2, 1)
    inp[:, 3, :] = sq
    return inp.reshape(NCORES * 4, N)


def _unpack(res):
    # uint8 [8*4096, 48] of LE 24-bit pairs -> int32 [8, 4096, 32]
    r = res.reshape(NCORES, N, NP2, 3).astype(np.int32)
    u = r[..., 0] | (r[..., 1] << 8) | (r[..., 2] << 16)
    out = np.empty((NCORES, N, NS), np.int32)
    out[..., 0::2] = u & 0xFFF
    out[..., 1::2] = u >> 12
    return out


def kernel(xyz, xyz_new=None):
    st = _CACHE if _CACHE else _init()
    jax, fn, sh = st["jax"], st["fn"], st["sh"]

    inp_dev = jax.device_put(_prep(xyz), sh)
    prev = st["prev"]
    if prev is None:
        prev = [
            jax.device_put(np.zeros(s, d), sh)
            for s, d in zip(st["out_shape"], st["out_dtype"])
        ]
    outs = fn(inp_dev, *prev)
    res = np.asarray(outs[0])
    st["prev"] = list(outs)
    return _unpack(res)


if __name__ == "__main__":
    rng = np.random.default_rng(0)
    xyz = rng.random((8, N, 3), dtype=np.float32)
    out = kernel(xyz)
    print(out.shape, out.dtype)


# revision 10
# speedup vs baseline: 12.0450x; 1.0382x over previous
"""Ball-query kernel for Trainium2 (Bass/Tile), 8 NeuronCores.

Problem: for each batch b (8 total) and each query point m (4096), return the
first 32 source indices n (in increasing n) with ||q_m - p_n||^2 < 0.2^2,
padding unused slots with the first valid index. Queries == sources (xyz).

Sharding: data-parallel over batch, one batch per core (8 cores).

Per-core algorithm (N=4096 queries x 4096 sources):
  - PE computes dot[m, n] = q_m . p_n per 128-query block (K=3 matmul).
  - DVE scalar_tensor_tensor: s = 2*dot - plus, where plus[m,n] = sq[m]+sq[n]
    (s == -d2 with bit-exact match to the reference's rounding order).
  - DVE STT: keys = (s > -r^2) * (4096 - n)  -> valid keys descending encode
    ascending indices; invalid -> 0.
  - 4 rounds of vector.max (top-8, descending) + match_replace to extract the
    32 largest keys = first 32 valid indices, in order.
  - Pad empty slots (key 0) with the first valid key; idx = 4096 - key.

Host I/O is the bottleneck (axon-tunneled PJRT), so the host->device payload
is a single [4, 4096] f32 tensor per core (xyzT rows + sq row; sqq and the
[128, N] broadcasts are materialized on-device via rearranged/stride-0 DMA +
iota), index pairs travel back packed as 24-bit triples in a uint8 tensor
(idx_even + 4096*idx_odd, exact in f32 since it is < 2^24), and the jitted
executable + donated output buffer are cached across kernel() calls.
"""

import numpy as np

N = 4096
NS = 32
R2 = 0.04
NCORES = 8
BLK = 128
NBLK = N // BLK   # 32
CH = 2048         # dve chunk
NCH = N // CH     # 2
MM = 512          # matmul free-dim per instruction (1 bank)
NP2 = NS // 2     # 16 packed index pairs per query
NB3 = 3 * NP2     # 48 output bytes per query

_CACHE = {}


def _build_bass():
    import concourse.bass as bass
    import concourse.mybir as mybir
    from concourse import bacc, tile

    Alu = mybir.AluOpType
    f32 = mybir.dt.float32
    i32 = mybir.dt.int32
    u8 = mybir.dt.uint8

    nc = bacc.Bacc(
        "TRN2", target_bir_lowering=False, debug=False, num_devices=NCORES
    )

    # rows 0:3 = xyzT; row 3 = sq (source order)
    inp_d = nc.dram_tensor("inp", [4, N], f32, kind="ExternalInput")
    out_d = nc.dram_tensor("out", [N, NB3], u8, kind="ExternalOutput")

    with tile.TileContext(nc) as tc:
        with (
            tc.tile_pool(name="const", bufs=1) as cpool,
            tc.tile_pool(name="psum", bufs=8, space="PSUM") as ppool,
            tc.tile_pool(name="work", bufs=2) as wpool,
            tc.tile_pool(name="small", bufs=3) as spool,
        ):
            xyzT_sb = cpool.tile([3, N], f32, tag="xyzT", name="xyzT_sb")
            nc.gpsimd.dma_start(xyzT_sb[:], inp_d.ap()[0:3, :])
            # sq broadcast along partitions via stride-0 DMA read
            sqrep = cpool.tile([128, N], f32, tag="sqrep", name="sqrep")
            nc.gpsimd.dma_start(
                sqrep[:], inp_d.ap()[3:4, :].partition_broadcast(128).squeeze(1)
            )
            # sqq[p, b] = sq[b*128 + p]: partition-major reread of the sq row
            sqq_sb = cpool.tile([128, NBLK], f32, tag="sqq", name="sqq_sb")
            nc.gpsimd.dma_start(
                sqq_sb[:], inp_d.ap()[3:4, :].rearrange("a (c p) -> p (a c)", p=128)
            )
            # ineg[p, n] = 4096 - n (exact in f32)
            ineg = cpool.tile([128, N], f32, tag="ineg", name="ineg")
            nc.gpsimd.iota(
                ineg[:],
                pattern=[[-1, N]],
                base=N,
                channel_multiplier=0,
                allow_small_or_imprecise_dtypes=True,
            )

            for b in range(NBLK):
                # plus[m, n] = sq_q[m] + sq_src[n]
                plus = wpool.tile([128, N], f32, tag="plus", name="plus")
                for c in range(NCH):
                    nc.vector.tensor_scalar(
                        plus[:, c * CH : (c + 1) * CH],
                        sqrep[:, c * CH : (c + 1) * CH],
                        sqq_sb[:, b : b + 1],
                        None,
                        Alu.add,
                    )

                keys = wpool.tile([128, N], f32, tag="keys", name="keys")
                keys2 = wpool.tile([128, N], f32, tag="keys2", name="keys2")

                for j in range(N // MM):
                    ps = ppool.tile([128, MM], f32, tag="ps", name="ps")
                    nc.tensor.matmul(
                        ps[:],
                        xyzT_sb[:, b * BLK : (b + 1) * BLK],
                        xyzT_sb[:, j * MM : (j + 1) * MM],
                        start=True,
                        stop=True,
                    )
                    # s = 2*dot - plus  (== -d2, exact)
                    nc.vector.scalar_tensor_tensor(
                        keys2[:, j * MM : (j + 1) * MM],
                        ps[:],
                        2.0,
                        plus[:, j * MM : (j + 1) * MM],
                        Alu.mult,
                        Alu.subtract,
                    )
                for c in range(NCH):
                    # keys = (s > -r2) * (4096 - n)
                    nc.vector.scalar_tensor_tensor(
                        keys[:, c * CH : (c + 1) * CH],
                        keys2[:, c * CH : (c + 1) * CH],
                        -R2,
                        ineg[:, c * CH : (c + 1) * CH],
                        Alu.is_gt,
                        Alu.mult,
                    )

                v8 = spool.tile([128, NS], f32, tag="v8", name="v8")
                nc.vector.max(v8[:, 0:8], keys[:])
                nc.vector.match_replace(keys2[:], v8[:, 0:8], keys[:], 0.0)
                nc.vector.max(v8[:, 8:16], keys2[:])
                nc.vector.match_replace(keys[:], v8[:, 8:16], keys2[:], 0.0)
                nc.vector.max(v8[:, 16:24], keys[:])
                nc.vector.match_replace(keys2[:], v8[:, 16:24], keys[:], 0.0)
                nc.vector.max(v8[:, 24:32], keys2[:])

                # pad empty slots (0) with first valid key, then pack index
                # pairs: idx_even + 4096*idx_odd = 2^24 + 4096 - k_e - 4096*k_o
                # (exact in f32: result <= 2^24 - 1)
                f8 = spool.tile([128, NS], f32, tag="f8", name="f8")
                nc.vector.tensor_scalar(f8[:], v8[:], 0.0, None, Alu.is_equal)
                t2 = spool.tile([128, NS], f32, tag="t2", name="t2")
                nc.vector.scalar_tensor_tensor(
                    t2[:], f8[:], v8[:, 0:1], v8[:], Alu.mult, Alu.add
                )
                idxf = spool.tile([128, NS], f32, tag="idxf", name="idxf")
                nc.vector.tensor_scalar(
                    idxf[:], t2[:], -1.0, float(N), Alu.mult, Alu.add
                )
                packed = spool.tile([128, NP2], i32, tag="packed", name="packed")
                nc.vector.scalar_tensor_tensor(
                    packed[:], idxf[:, NP2:NS], 4096.0, idxf[:, 0:NP2],
                    Alu.mult, Alu.add,
                )
                # byte-plane output (LE): cols 0:32 = lo16 pairs, 32:48 = hi bytes
                src_b = packed[:].bitcast(u8).rearrange("p (j k) -> p j k", k=4)
                orow = out_d.ap()[b * BLK : (b + 1) * BLK, :]
                nc.sync.dma_start(
                    orow[:, 0 : 2 * NP2].rearrange("p (j k) -> p j k", k=2),
                    src_b[:, :, 0:2],
                )
                nc.sync.dma_start(orow[:, 2 * NP2 : NB3], src_b[:, :, 2])

    nc.compile()
    return nc


def _init():
    import jax
    from jax.sharding import Mesh, PartitionSpec, NamedSharding

    try:
        from jax.experimental.shard_map import shard_map
    except ImportError:
        from jax import shard_map
    import concourse.mybir as mybir
    from concourse.bass2jax import (
        _bass_exec_p,
        install_neuronx_cc_hook,
        partition_id_tensor,
    )

    install_neuronx_cc_hook()
    nc = _build_bass()

    partition_name = (
        nc.partition_id_tensor.name if nc.partition_id_tensor else None
    )
    in_names, out_names, out_avals = [], [], []
    for alloc in nc.m.functions[0].allocations:
        if not isinstance(alloc, mybir.MemoryLocationSet):
            continue
        name = alloc.memorylocations[0].name
        if alloc.kind == "ExternalInput":
            if name != partition_name:
                in_names.append(name)
        elif alloc.kind == "ExternalOutput":
            shape = tuple(alloc.tensor_shape)
            dtype = mybir.dt.np(alloc.dtype)
            out_names.append(name)
            out_avals.append(jax.core.ShapedArray(shape, dtype))
    n_params = len(in_names)
    n_outs = len(out_avals)
    in_names_full = in_names + out_names + (
        [partition_name] if partition_name else []
    )
    donate = tuple(range(n_params, n_params + n_outs))

    def _body(*args):
        operands = list(args)
        if partition_name is not None:
            operands.append(partition_id_tensor())
        outs = _bass_exec_p.bind(
            *operands,
            out_avals=tuple(out_avals),
            in_names=tuple(in_names_full),
            out_names=tuple(out_names),
            lowering_input_output_aliases=(),
            sim_require_finite=True,
            sim_require_nnan=True,
            nc=nc,
        )
        return tuple(outs)

    devices = jax.devices()[:NCORES]
    mesh = Mesh(np.asarray(devices), ("core",))
    sh = NamedSharding(mesh, PartitionSpec("core"))
    in_specs = (PartitionSpec("core"),) * (n_params + n_outs)
    out_specs = (PartitionSpec("core"),) * n_outs
    fn = jax.jit(
        shard_map(
            _body, mesh=mesh, in_specs=in_specs, out_specs=out_specs,
            check_rep=False,
        ),
        donate_argnums=donate,
        keep_unused=True,
    )
    _CACHE.update(
        jax=jax, fn=fn, sh=sh, prev=None,
        out_shape=[(NCORES * N, NB3)], out_dtype=[np.uint8],
    )
    return _CACHE


def _prep(xyz):
    # [8, 4096, 3] -> concat of per-core [4, 4096]: xyzT rows + sq row
    xyz = np.ascontiguousarray(np.asarray(xyz, dtype=np.float32))
    x, y, z = xyz[..., 0], xyz[..., 1], xyz[..., 2]
    sq = (x * x + y * y) + z * z                      # [8, 4096] reference order
    inp = np.empty((NCORES, 4, N), np.float32)
    inp[:, 0:3, :] = xyz.transpose(0, 2, 1)
    inp[:, 3, :] = sq
    return inp.reshape(NCORES * 4, N)


def _unpack(res):
    # uint8 [8*4096, 48]: cols 0:32 = LE lo16 of pair j (idx_j + 4096*idx_{j+16}),
    # cols 32:48 = hi byte -> int32 [8, 4096, 32]
    lo = res[:, 0 : 2 * NP2].view(np.uint16)
    hi = res[:, 2 * NP2 : NB3]
    out = np.empty((NCORES, N, NS), np.int32)
    o = out.reshape(NCORES * N, NS)
    o[:, 0:NP2] = lo & 0xFFF
    o[:, NP2:NS] = (lo >> 12) | (hi.astype(np.uint16) << 4)
    return out


def kernel(xyz, xyz_new=None):
    st = _CACHE if _CACHE else _init()
    jax, fn, sh = st["jax"], st["fn"], st["sh"]

    inp_dev = jax.device_put(_prep(xyz), sh)
    prev = st["prev"]
    if prev is None:
        prev = [
            jax.device_put(np.zeros(s, d), sh)
            for s, d in zip(st["out_shape"], st["out_dtype"])
        ]
    outs = fn(inp_dev, *prev)
    res = np.asarray(outs[0])
    st["prev"] = list(outs)
    return _unpack(res)


if __name__ == "__main__":
    rng = np.random.default_rng(0)
    xyz = rng.random((8, N, 3), dtype=np.float32)
    out = kernel(xyz)
    print(out.shape, out.dtype)
